# revision 27
# baseline (speedup 1.0000x reference)
"""Causal multi-head self-attention on 8 Trainium2 NeuronCores.

Problem: B=4, T=2048, C=1024, H=16 heads (d=64), fp32 in/out.
    q/k/v = x @ W{q,k,v}.T + b;  S = causal softmax(q k^T / sqrt(d));  y = (S v) @ Wo.T + bo

Sharding (8 cores): 2-D  (batch x head-group).
    core c -> batch b = c // 2, head-group g = c % 2 (8 heads / 512 features).
    Each core computes its batch's attention for its 8 heads plus the partial
    output projection against Wo[:, 512g:512g+512]; the host sums the two
    partials per batch and adds bo.

Device kernel (per core, identical SPMD program, Bass/Tile), v2:
  - All inputs are uploaded in SBUF-image DRAM layouts so each tensor loads
    with a couple of large, perfectly-contiguous DMAs (2-8KB runs per
    partition) instead of many 256B-run descriptors; loads are issued from
    BOTH HWDGE rings (sync + scalar queues) so issue serialization doesn't
    delay the critical x0+Wq pieces.
  - The attention is one flat, software-pipelined stream over all
    (chunk, head-pair, j-tile) steps: the S-pair + exp of step i+2 issue
    before the PV pair of step i, so the ACT engine (the attention-phase
    pacer) never drains across head-pair or chunk boundaries.
  - Projection / output-projection matmuls are metered into the stream
    ("pumping") through a FIFO of generators, 2 matmuls per yield.
  - Final chunk's output projection is split 3+1: head-pairs 0-2 are
    projected to bf16 partials while head-pair 3's attention still runs; the
    tail is then one matmul + an identity-matmul partial-add per piece with
    the PSUM->SBUF evacuation on the (idle) scalar engine.
"""

import math
import os

import numpy as np

os.environ.setdefault("JAX_COMPILATION_CACHE_DIR", "/tmp/jax_comp_cache")

B, T, C, H = 4, 2048, 1024, 16
D = C // H  # 64
NCORES = 8
GROUPS = 2  # head-groups (tensor parallel dimension)
HG = H // GROUPS  # heads per core = 8
CG = C // GROUPS  # features per core = 512
SCALE = 1.0 / math.sqrt(D)
P = 128
TCH = 512  # query chunk / matmul free dim
NTCH = T // TCH  # 4
NHP = CG // P  # 4 head-pairs per core

_MODULE_CACHE = {}


def _build_module(mm_fast=True):
    import concourse.bass as bass  # noqa: F401
    import concourse.mybir as mybir
    import concourse.tile as tile
    from concourse import bacc

    f32 = mybir.dt.float32
    bdt = mybir.dt.bfloat16  # q/k/proj-weight/x/ot dtype
    edt = mybir.dt.float32r  # E and V dtype (PV matmul operands)
    Exp = mybir.ActivationFunctionType.Exp

    nc = bacc.Bacc(None, target_bir_lowering=False)

    # SBUF-image layouts (host pre-permuted):
    xt_im = nc.dram_tensor("xt_im", [P, NTCH, 8, TCH], bdt, kind="ExternalInput")
    wq_im = nc.dram_tensor("wq_im", [P, NHP, 8, P], bdt, kind="ExternalInput")
    wk_im = nc.dram_tensor("wk_im", [P, NHP, 8, P], bdt, kind="ExternalInput")
    wv_im = nc.dram_tensor("wv_im", [P, 8, CG], bdt, kind="ExternalInput")
    wot_im = nc.dram_tensor("wot_im", [P, NHP, C], bdt, kind="ExternalInput")
    bias_im = nc.dram_tensor("bias_im", [P, 2 * NHP + CG], f32, kind="ExternalInput")
    out = nc.dram_tensor("out", [T, C], bdt, kind="ExternalOutput")
    out_ap = out.ap()

    with tile.TileContext(nc) as tc:
        with (
            tc.tile_pool(name="persist", bufs=1) as persist,
            tc.tile_pool(name="smalls", bufs=1) as smalls,
            tc.tile_pool(name="qpool", bufs=2) as qpool,
            tc.tile_pool(name="xp", bufs=2) as xp,
            tc.tile_pool(name="otp", bufs=3) as otp,
            tc.tile_pool(name="ep", bufs=10) as ep,
            tc.tile_pool(name="op", bufs=4) as op,
            tc.tile_pool(name="oap", bufs=8) as oap,
            tc.tile_pool(name="npool", bufs=3) as npool,
            tc.tile_pool(name="psS", bufs=2, space="PSUM") as psS,
            tc.tile_pool(name="psO", bufs=2, space="PSUM") as psO,
            tc.tile_pool(name="ps3", bufs=2, space="PSUM") as ps3p,
        ):
            # per-(chunk, slice) tiles: Tile's dependency tracking is
            # coarse-grained, so slices written by different projection
            # chains must be separate tiles or chunk-c attention reads
            # falsely serialize on later slices' writes.
            kT_t = {}  # (c, hp) -> [feature-partition, token]
            vx_t = {}  # (c, tt) -> [token-partition, head, d+1]
            for c in range(NTCH):
                for sl in range(NHP):
                    kT_t[(c, sl)] = persist.tile([P, TCH], bdt, name=f"kT{c}_{sl}")
                    vx_t[(c, sl)] = persist.tile(
                        [P, HG, D + 1], edt, name=f"vx{c}_{sl}"
                    )

            wqts = persist.tile([P, NHP, 8, P], bdt, name="wqts")
            wkts = persist.tile([P, NHP, 8, P], bdt, name="wkts")
            wvts = persist.tile([P, 8, CG], bdt, name="wvts")
            wots = persist.tile([P, NHP, C], bdt, name="wots")
            biases = smalls.tile([P, 2 * NHP + CG], f32, name="biases")
            bqs = biases[:, 0:NHP]
            bks = biases[:, NHP : 2 * NHP]
            bvbs = biases[:, 2 * NHP :]

            xtt0 = xp.tile([P, 8, TCH], bdt, tag="xtt", name="xtt0")

            # ---- input loads: large image DMAs in 3 phases so the critical
            # x0+Wq pieces get the full HBM bandwidth (queued DMAs round-robin
            # at packet granularity, so without phase barriers everything
            # finishes together at the end).  Phase barriers are tiny
            # SBUF->DRAM reads spanning both halves of the previous phase:
            # the barrier DMA instruction blocks its HWDGE ring until they
            # land.  Barrier target rows of `out` are rewritten later by the
            # real output projection.
            # Input loads, strictly phased: concurrently-queued DMAs
            # round-robin at packet granularity across all SDMA engines and
            # finish together, so each subsequent load is gated behind the
            # previous phase with a tiny DVE copy into its destination tile
            # (a WAW dependency both the scheduler and hardware respect).
            # Phase 0 is the minimal first-projection set: bias + x0a + wq-a.
            # ---- warmup + mask/identity setup: emitted BEFORE the DMA gate
            # chain so these DVE ops aren't stuck behind gate copies that
            # wait on DMA completions (the DVE queue is FIFO).
            wu = smalls.tile([P, P], bdt, name="wu")
            nc.vector.memset(wu, 0.125)
            wups = ps3p.tile([P, 64], f32, tag="pso3", name="wups")
            for i in range(100):
                nc.tensor.matmul(
                    wups, wu, wu[:, 0:64], start=(i == 0), stop=(i == 99)
                )
            wue = smalls.tile([P, 1], f32, name="wue")
            nc.scalar.activation(wue, wups[:, 0:1], Exp, scale=SCALE)

            # triangular causal mask for diagonal 128x128 blocks:
            # keep (j, q) where q >= j  (partition j, free q)
            cmask = smalls.tile([P, P], edt, name="cmask")
            nc.vector.tensor_scalar(
                cmask, wu, 0.0, 1.0, mybir.AluOpType.mult, mybir.AluOpType.add
            )
            nc.gpsimd.affine_select(
                out=cmask,
                in_=cmask,
                compare_op=mybir.AluOpType.is_ge,
                fill=0.0,
                base=0,
                pattern=[[1, P]],
                channel_multiplier=-1,
            )
            # identity matrix (bf16) for the final partial-sum matmul-add
            ident = smalls.tile([P, P], bdt, name="ident")
            nc.vector.tensor_scalar(
                ident, wu, 0.0, 1.0, mybir.AluOpType.mult, mybir.AluOpType.add
            )
            nc.gpsimd.affine_select(
                out=ident,
                in_=ident,
                compare_op=mybir.AluOpType.is_equal,
                fill=0.0,
                base=0,
                pattern=[[1, P]],
                channel_multiplier=-1,
            )

            def gate(dst, src):
                nc.vector.tensor_copy(dst, src)

            # phase 0: bias + x0 | wq (the first q-projection's inputs)
            nc.sync.dma_start(biases, bias_im.ap())
            nc.sync.dma_start(xtt0[:, 0:4, :], xt_im.ap()[:, 0, 0:4, :])
            nc.sync.dma_start(xtt0[:, 4:8, :], xt_im.ap()[:, 0, 4:8, :])
            nc.scalar.dma_start(wqts[:, 0:2], wq_im.ap()[:, 0:2])
            nc.scalar.dma_start(wqts[:, 2:4], wq_im.ap()[:, 2:4])
            # phase 1: wk
            gate(wkts[0:1, 0:2, 0, 0:8], wqts[0:1, 2:4, 0, 0:8])
            nc.scalar.dma_start(wkts[:, 0:2], wk_im.ap()[:, 0:2])
            gate(wkts[0:1, 2:4, 0, 0:8], xtt0[0:1, 5:7, 0:8])
            nc.scalar.dma_start(wkts[:, 2:4], wk_im.ap()[:, 2:4])
            # phase 2: wv
            gate(wvts[0:1, 0:2, 0:8], wkts[0:1, 0:2, 0, 0:8])
            nc.scalar.dma_start(wvts[:, 0:4, :], wv_im.ap()[:, 0:4, :])
            gate(wvts[0:1, 4:6, 0:8], wkts[0:1, 2:4, 0, 0:8])
            nc.scalar.dma_start(wvts[:, 4:8, :], wv_im.ap()[:, 4:8, :])
            # phase 3: wot
            gate(wots[0:1, 0:2, 0:8], wvts[0:1, 3:5, 0:8])
            nc.scalar.dma_start(wots, wot_im.ap())

            qT_cur = {}  # chunk -> qT tile (2-buf rotation)
            xtt_cur = {0: xtt0}
            ot_t = {}

            # ---- pump queues: generators emitting proj/outproj matmuls in
            # ~2-matmul units so they weave between exp-paced attention ops.
            # `pending` (urgent: q/k/v projections - the attention stream
            # stalls without them) drains before `filler` (output
            # projections - pure latency-tolerant PE filler).
            pending = []
            filler = []

            def pump2(n):
                while n > 0:
                    q = pending if pending else filler
                    if not q:
                        return
                    tag, g = q[0]
                    try:
                        next(g)
                        n -= 1
                    except StopIteration:
                        q.pop(0)

            def drain_all():
                while pending or filler:
                    pump2(100)

            def ensure_done(tag):
                """Emission barrier: a consumer of `tag`'s outputs is about
                to be emitted; Tile tracks dependencies by emission order,
                so the producer must be fully emitted first."""
                while any(t == tag for t, _ in pending):
                    pump2(4)

            def qk_gen(c, sl):
                """q and k feature-slice sl of chunk c; yields every 2 mms."""
                qT_cur[(c, sl)] = qpool.tile(
                    [P, TCH], bdt, tag=f"qT{sl}", name=f"qT{c}_{sl}"
                )
                xtt = xtt_cur[c]
                psq = ps3p.tile([P, TCH], f32, tag="pso3", name=f"psq{c}_{sl}")
                for cs in range(8):
                    nc.tensor.matmul(
                        psq,
                        wqts[:, sl, cs, :],
                        xtt[:, cs, :],
                        start=(cs == 0),
                        stop=(cs == 7),
                    )
                    if cs % 2 == 1:
                        yield
                with tc.high_priority(offset=300_000):
                    nc.vector.tensor_scalar_add(
                        qT_cur[(c, sl)], psq, bqs[:, sl : sl + 1]
                    )
                psk = ps3p.tile([P, TCH], f32, tag="pso3", name=f"psk{c}_{sl}")
                for cs in range(8):
                    nc.tensor.matmul(
                        psk,
                        wkts[:, sl, cs, :],
                        xtt[:, cs, :],
                        start=(cs == 0),
                        stop=(cs == 7),
                    )
                    if cs % 2 == 1:
                        yield
                with tc.high_priority(offset=300_000):
                    nc.vector.tensor_scalar_add(
                        kT_t[(c, sl)], psk, bks[:, sl : sl + 1]
                    )

            def v_gen(c, sl):
                """v token-tile sl of chunk c; yields every 2 mms."""
                # ones column of v_ext (softmax-denominator trick);
                # memset can't write float32r -> DVE 0*x + 1.
                nc.vector.tensor_scalar(
                    vx_t[(c, sl)][:, :, D],
                    bvbs[:, 0:HG],
                    0.0,
                    1.0,
                    mybir.AluOpType.mult,
                    mybir.AluOpType.add,
                )
                xtt = xtt_cur[c]
                psv = ps3p.tile([P, CG], f32, tag="pso3", name=f"psv{c}_{sl}")
                for cs in range(8):
                    nc.tensor.matmul(
                        psv,
                        xtt[:, cs, P * sl : P * (sl + 1)],
                        wvts[:, cs, :],
                        start=(cs == 0),
                        stop=(cs == 7),
                    )
                    if cs % 2 == 1:
                        yield
                with tc.high_priority(offset=300_000):
                    nc.vector.tensor_add(
                        vx_t[(c, sl)][:, :, 0:D],
                        psv.rearrange("p (h d) -> p h d", d=D),
                        bvbs.rearrange("p (h d) -> p h d", d=D),
                    )

            def outproj_gen(ic, plo=0, phi=2 * (TCH // P)):
                otn = [ot_t[(ic, hp)] for hp in range(NHP)]
                for piece in range(plo, phi):
                    tt, mi = piece // 2, piece % 2
                    trow = TCH * ic + P * tt
                    if True:
                        msl = slice(TCH * mi, TCH * (mi + 1))
                        pso3 = ps3p.tile(
                            [P, TCH], f32, tag="pso3", name=f"ps3{ic}_{tt}_{mi}"
                        )
                        for hp in range(NHP):
                            nc.tensor.matmul(
                                pso3,
                                otn[hp][:, P * tt : P * (tt + 1)],
                                wots[:, hp, msl],
                                start=(hp == 0),
                                stop=(hp == NHP - 1),
                            )
                            if hp % 2 == 1:
                                yield
                        osb = op.tile(
                            [P, TCH], bdt, tag="osb", name=f"ob{ic}_{tt}_{mi}"
                        )
                        nc.vector.tensor_copy(osb, pso3)
                        nc.sync.dma_start(out_ap[trow : trow + P, msl], osb)

            obA = []

            def outproj_final_A():
                """Final chunk head-pairs 0-2 -> bf16 partials (runs during
                the last head-pair's attention)."""
                ic = NTCH - 1
                for tt in range(TCH // P):
                    for mi in range(C // TCH):
                        msl = slice(TCH * mi, TCH * (mi + 1))
                        pso3 = ps3p.tile(
                            [P, TCH], f32, tag="pso3", name=f"psA{tt}_{mi}"
                        )
                        for hp in (0, 1, 2):
                            nc.tensor.matmul(
                                pso3,
                                ot_t[(ic, hp)][:, P * tt : P * (tt + 1)],
                                wots[:, hp, msl],
                                start=(hp == 0),
                                stop=(hp == 2),
                            )
                            if hp == 1:
                                yield
                        ob = oap.tile(
                            [P, TCH], bdt, tag="obA", name=f"obA{2 * tt + mi}"
                        )
                        nc.vector.tensor_copy(ob, pso3)
                        obA.append(ob)
                        yield

            def outproj_final_B():
                """Tail: head-pair 3 matmul + identity partial-add; psum
                evacuation on the scalar engine (idle at the tail)."""
                ic = NTCH - 1
                otn3 = ot_t.pop((ic, 3))
                with tc.high_priority(offset=450_000):
                    for tt in range(TCH // P):
                        trow = TCH * ic + P * tt
                        for mi in range(C // TCH):
                            i = 2 * tt + mi
                            msl = slice(TCH * mi, TCH * (mi + 1))
                            pso3 = ps3p.tile(
                                [P, TCH], f32, tag="pso3", name=f"psB{i}"
                            )
                            nc.tensor.matmul(
                                pso3, ident, obA[i], start=True, stop=False
                            )
                            nc.tensor.matmul(
                                pso3,
                                otn3[:, P * tt : P * (tt + 1)],
                                wots[:, 3, msl],
                                start=False,
                                stop=True,
                            )
                            osb = op.tile(
                                [P, TCH], bdt, tag="osb", name=f"obf{i}"
                            )
                            nc.scalar.copy(osb, pso3)
                            nc.sync.dma_start(out_ap[trow : trow + P, msl], osb)

            def normalize(c, hp, oraw, final=False):
                # rows 0..63 are O^T, row 64 the softmax sums.
                # partition_broadcast only reads physical partition 0
                # (base-64 APs return garbage on HW): DMA-hop the row.
                off = 450_000 if final else 0
                otn = ot_t[(c, hp)]
                for h01 in range(2):
                    ow = oraw[h01]
                    stmp = npool.tile(
                        [1, TCH], f32, tag="stmp", name=f"st{c}_{hp}_{h01}"
                    )
                    rb = npool.tile(
                        [D, TCH], f32, tag="rb", name=f"rb{c}_{hp}_{h01}"
                    )
                    if final:
                        with tc.high_priority(offset=off):
                            nc.sync.dma_start(stmp, ow[D : D + 1, :])
                            nc.gpsimd.partition_broadcast(rb, stmp)
                            nc.vector.reciprocal_approx_fast(rb, rb)
                    else:
                        nc.sync.dma_start(stmp, ow[D : D + 1, :])
                        nc.gpsimd.partition_broadcast(rb, stmp)
                        nc.vector.reciprocal_approx_fast(rb, rb)
                    if h01 == 0:
                        if final:
                            with tc.high_priority(offset=off):
                                nc.vector.tensor_mul(otn[0:D, :], ow[0:D, :], rb)
                        else:
                            nc.vector.tensor_mul(otn[0:D, :], ow[0:D, :], rb)
                    else:
                        tmpn = npool.tile(
                            [D, TCH], bdt, tag="tmpn", name=f"tn{c}_{hp}"
                        )
                        if final:
                            with tc.high_priority(offset=off):
                                nc.vector.tensor_mul(tmpn, ow[0:D, :], rb)
                                nc.sync.dma_start(otn[D:P, :], tmpn)
                        else:
                            nc.vector.tensor_mul(tmpn, ow[0:D, :], rb)
                            nc.sync.dma_start(otn[D:P, :], tmpn)

            # ---- flat software-pipelined attention over all j-steps.
            steps = [
                (c, hp, jt)
                for c in range(NTCH)
                for hp in range(NHP)
                for jt in range(4 * (c + 1))
            ]
            LAG = 2
            ee_of = {}
            po_of = {}

            def passA(i):
                c, hp, jt = steps[i]
                if jt == 0:
                    ensure_done(f"qk{c}_{hp}")
                cj, lj = jt // 4, jt % 4
                r = jt - 4 * c
                lo = 0 if r <= 0 else P * r
                psp = psS.tile(
                    [P, 2, TCH], f32, tag="psp", name=f"psp{c}_{hp}_{jt}"
                )
                with tc.high_priority(offset=1_000_000):
                    for h01 in range(2):
                        pb = 64 * h01
                        nc.tensor.matmul(
                            psp[:, h01, lo:],
                            kT_t[(cj, hp)][pb : pb + D, P * lj : P * (lj + 1)],
                            qT_cur[(c, hp)][pb : pb + D, lo:],
                            start=True,
                            stop=True,
                        )
                    ee = ep.tile(
                        [P, 2, TCH], edt, tag="ee", name=f"ee{c}_{hp}_{jt}"
                    )
                    if r <= 0:
                        nc.scalar.activation(ee, psp, Exp, scale=SCALE)
                    else:
                        nc.scalar.activation(
                            ee[:, :, lo:], psp[:, :, lo:], Exp, scale=SCALE
                        )
                    if r >= 0:
                        bsl = slice(P * r, P * (r + 1))
                        for h01 in range(2):
                            nc.vector.tensor_mul(
                                ee[:, h01, bsl], ee[:, h01, bsl], cmask
                            )
                ee_of[i] = ee

            def passB(j):
                c, hp, jt = steps[j]
                njt = 4 * (c + 1)
                if jt == 0:
                    ot_t[(c, hp)] = otp.tile(
                        [P, TCH], bdt, tag=f"ot{hp}", name=f"ot{c}_{hp}"
                    )
                    po_of[(c, hp)] = (
                        psO.tile([P, TCH], f32, tag="ps_o", name=f"poe{c}_{hp}"),
                        psO.tile([P, TCH], f32, tag="ps_o", name=f"poo{c}_{hp}"),
                    )
                ps_os = po_of[(c, hp)]
                ee = ee_of.pop(j)
                cj, lj = jt // 4, jt % 4
                if cj == c:
                    ensure_done(f"v{c}_{lj}")
                lo = max(0, P * (jt - 4 * c))
                with tc.high_priority(offset=500_000):
                    for h01 in range(2):
                        nc.tensor.matmul(
                            ps_os[h01][0 : D + 1, lo:],
                            vx_t[(cj, lj)][:, 2 * hp + h01, :],
                            ee[:, h01, lo:],
                            start=(jt == 0),
                            stop=(jt == njt - 1),
                        )
                if jt == njt - 1:
                    del po_of[(c, hp)]
                    oraw = []
                    for h01 in range(2):
                        ow = npool.tile(
                            [D + 1, TCH],
                            f32,
                            tag=f"oraw{h01}",
                            name=f"or{c}_{hp}_{h01}",
                        )
                        with tc.high_priority(offset=500_000):
                            nc.vector.tensor_copy(ow, ps_os[h01][0 : D + 1, :])
                        oraw.append(ow)
                    final = c == NTCH - 1 and hp == NHP - 1
                    if final:
                        # held-back output-projection pieces: pure PE filler
                        # released at the tail so the PE has work while the
                        # final normalize's latency chain runs.
                        filler.append(("out2b", outproj_gen(NTCH - 2, 5, 8)))
                    normalize(c, hp, oraw, final=final)
                    if hp == NHP - 1 and c < NTCH - 1:
                        phi = 5 if c == NTCH - 2 else 8
                        filler.append((f"out{c}", outproj_gen(c, 0, phi)))
                    if c == NTCH - 1 and hp == 2:
                        filler.append(("outA", outproj_final_A()))

            # proj generator launch schedule, keyed by global head-pair
            # phase ph = 4c + hp.  Each qk(c, sl) is queued ~2 phases before
            # its attention phase needs it: early enough that the pump
            # finishes it in time, late enough that chunks 2-3 (exp-heavy)
            # keep their own share of PE filler.
            schedule = {0: [("qk0_2", qk_gen, (0, 2))], 1: [("qk0_3", qk_gen, (0, 3))]}
            for c in range(1, NTCH):
                schedule.setdefault(4 * c - 2, []).extend(
                    [
                        (f"qk{c}_0", qk_gen, (c, 0)),
                        (f"v{c}_0", v_gen, (c, 0)),
                        (f"v{c}_1", v_gen, (c, 1)),
                    ]
                )
                schedule.setdefault(4 * c - 1, []).extend(
                    [
                        (f"qk{c}_1", qk_gen, (c, 1)),
                        (f"v{c}_2", v_gen, (c, 2)),
                        (f"v{c}_3", v_gen, (c, 3)),
                    ]
                )
                schedule.setdefault(4 * c, []).append((f"qk{c}_2", qk_gen, (c, 2)))
                schedule.setdefault(4 * c + 1, []).append(
                    (f"qk{c}_3", qk_gen, (c, 3))
                )

            # proj(0) slice 0 inline; the S/exp stream starts immediately
            # after so the scheduler places it at the head of the PE queue.
            for _ in qk_gen(0, 0):
                pass
            for sl in range(NHP):
                pending.append((f"v0_{sl}", v_gen(0, sl)))
            pending.append(("qk0_1", qk_gen(0, 1)))

            for i in range(len(steps) + LAG):
                if i < len(steps):
                    c, hp, jt = steps[i]
                    if jt == 0:
                        ph = 4 * c + hp
                        if hp == 0 and c + 1 < NTCH:
                            # prefetch next chunk's x (gated behind wk for
                            # chunk 1 so it can't steal phase-1 bandwidth)
                            xtt = xp.tile(
                                [P, 8, TCH], bdt, tag="xtt", name=f"xtt{c + 1}"
                            )
                            if c == 0:
                                nc.vector.tensor_copy(
                                    xtt[0:1, 0:2, 0:8], wkts[0:1, 2:4, 0, 0:8]
                                )
                            nc.sync.dma_start(xtt, xt_im.ap()[:, c + 1])
                            xtt_cur[c + 1] = xtt
                        for tag, fn, args in schedule.pop(ph, []):
                            pending.append((tag, fn(*args)))
                    passA(i)
                    pump2(2)
                if i >= LAG:
                    passB(i - LAG)
                    pump2(2)

            drain_all()
            outproj_final_B()

    nc.compile()
    return nc


def get_module(mm_fast=True):
    key = bool(mm_fast)
    if key not in _MODULE_CACHE:
        _MODULE_CACHE[key] = _build_module(key)
    return _MODULE_CACHE[key]


def make_in_maps(x, Wq, bq, Wk, bk, Wv, bv, Wo, bo):
    import ml_dtypes

    bf16 = ml_dtypes.bfloat16
    x = np.asarray(x, dtype=np.float32)
    Wq = np.asarray(Wq, dtype=np.float32)
    Wk = np.asarray(Wk, dtype=np.float32)
    Wv = np.asarray(Wv, dtype=np.float32)
    Wo = np.asarray(Wo, dtype=np.float32)
    bq = np.asarray(bq, dtype=np.float32)
    bk = np.asarray(bk, dtype=np.float32)
    bv = np.asarray(bv, dtype=np.float32)

    in_maps = []
    for core in range(NCORES):
        b, g = core // GROUPS, core % GROUPS
        gs = slice(CG * g, CG * (g + 1))
        # x image: [p, chunk, cs, t']
        xim = (
            x[b].T.reshape(8, P, NTCH, TCH).transpose(1, 2, 0, 3)
        )  # [128, 4, 8, 512]
        # wq/wk image: [p, j4, cs, j']
        wqi = Wq[gs, :].T.reshape(8, P, NHP, P).transpose(1, 2, 0, 3)
        wki = Wk[gs, :].T.reshape(8, P, NHP, P).transpose(1, 2, 0, 3)
        # wv image: [p, cs, j]
        wvi = Wv[gs, :].T.reshape(8, P, CG).transpose(1, 0, 2)
        # wot image: [p, hp, m]
        woi = Wo[:, gs].T.reshape(NHP, P, C).transpose(1, 0, 2)
        bias = np.concatenate(
            [
                bq[gs].reshape(NHP, P).T,
                bk[gs].reshape(NHP, P).T,
                np.broadcast_to(bv[gs][None, :], (P, CG)),
            ],
            axis=1,
        )
        in_maps.append(
            {
                "xt_im": np.ascontiguousarray(xim).astype(bf16),
                "wq_im": np.ascontiguousarray(wqi).astype(bf16),
                "wk_im": np.ascontiguousarray(wki).astype(bf16),
                "wv_im": np.ascontiguousarray(wvi).astype(bf16),
                "wot_im": np.ascontiguousarray(woi).astype(bf16),
                "bias_im": np.ascontiguousarray(bias),
            }
        )
    return in_maps


def combine_results(results, bo):
    bo = np.asarray(bo, dtype=np.float32)
    out = np.empty((B, T, C), dtype=np.float32)
    for b in range(B):
        out[b] = (
            results[GROUPS * b]["out"].astype(np.float32)
            + results[GROUPS * b + 1]["out"].astype(np.float32)
            + bo[None, :]
        )
    return out


def kernel(**inputs):
    from concourse.bass_utils import run_bass_kernel_spmd

    nc = get_module(mm_fast=True)
    in_maps = make_in_maps(
        inputs["x"],
        inputs["Wq"],
        inputs["bq"],
        inputs["Wk"],
        inputs["bk"],
        inputs["Wv"],
        inputs["bv"],
        inputs["Wo"],
        inputs["bo"],
    )
    res = run_bass_kernel_spmd(nc, in_maps, core_ids=list(range(NCORES)))
    return combine_results(res.results, inputs["bo"])


# revision 29
# speedup vs baseline: 1.0252x; 1.0252x over previous
"""Causal multi-head self-attention on 8 Trainium2 NeuronCores.

Problem: B=4, T=2048, C=1024, H=16 heads (d=64), fp32 in/out.
    q/k/v = x @ W{q,k,v}.T + b;  S = causal softmax(q k^T / sqrt(d));  y = (S v) @ Wo.T + bo

Sharding (8 cores): 2-D  (batch x head-group).
    core c -> batch b = c // 2, head-group g = c % 2 (8 heads / 512 features).
    Each core computes its batch's attention for its 8 heads plus the partial
    output projection against Wo[:, 512g:512g+512]; the host sums the two
    partials per batch and adds bo.

Device kernel (per core, identical SPMD program, Bass/Tile), v2:
  - All inputs are uploaded in SBUF-image DRAM layouts so each tensor loads
    with a couple of large, perfectly-contiguous DMAs (2-8KB runs per
    partition) instead of many 256B-run descriptors; loads are issued from
    BOTH HWDGE rings (sync + scalar queues) so issue serialization doesn't
    delay the critical x0+Wq pieces.
  - The attention is one flat, software-pipelined stream over all
    (chunk, head-pair, j-tile) steps: the S-pair + exp of step i+2 issue
    before the PV pair of step i, so the ACT engine (the attention-phase
    pacer) never drains across head-pair or chunk boundaries.
  - Projection / output-projection matmuls are metered into the stream
    ("pumping") through a FIFO of generators, 2 matmuls per yield.
  - Final chunk's output projection is split 3+1: head-pairs 0-2 are
    projected to bf16 partials while head-pair 3's attention still runs; the
    tail is then one matmul + an identity-matmul partial-add per piece with
    the PSUM->SBUF evacuation on the (idle) scalar engine.
"""

import math
import os

import numpy as np

os.environ.setdefault("JAX_COMPILATION_CACHE_DIR", "/tmp/jax_comp_cache")

B, T, C, H = 4, 2048, 1024, 16
D = C // H  # 64
NCORES = 8
GROUPS = 2  # head-groups (tensor parallel dimension)
HG = H // GROUPS  # heads per core = 8
CG = C // GROUPS  # features per core = 512
SCALE = 1.0 / math.sqrt(D)
P = 128
TCH = 512  # query chunk / matmul free dim
NTCH = T // TCH  # 4
NHP = CG // P  # 4 head-pairs per core

_MODULE_CACHE = {}


def _build_module(mm_fast=True):
    import concourse.bass as bass  # noqa: F401
    import concourse.mybir as mybir
    import concourse.tile as tile
    from concourse import bacc

    f32 = mybir.dt.float32
    bdt = mybir.dt.bfloat16  # q/k/proj-weight/x/ot dtype
    edt = mybir.dt.float32r  # E and V dtype (PV matmul operands)
    Exp = mybir.ActivationFunctionType.Exp

    nc = bacc.Bacc(None, target_bir_lowering=False)

    # SBUF-image layouts (host pre-permuted):
    xt_im = nc.dram_tensor("xt_im", [P, NTCH, 8, TCH], bdt, kind="ExternalInput")
    wq_im = nc.dram_tensor("wq_im", [P, NHP, 8, P], bdt, kind="ExternalInput")
    wk_im = nc.dram_tensor("wk_im", [P, NHP, 8, P], bdt, kind="ExternalInput")
    wv_im = nc.dram_tensor("wv_im", [P, 8, CG], bdt, kind="ExternalInput")
    wot_im = nc.dram_tensor("wot_im", [P, NHP, C], bdt, kind="ExternalInput")
    bias_im = nc.dram_tensor("bias_im", [P, 2 * NHP + CG], f32, kind="ExternalInput")
    out = nc.dram_tensor("out", [T, C], bdt, kind="ExternalOutput")
    out_ap = out.ap()

    with tile.TileContext(nc) as tc:
        with (
            tc.tile_pool(name="persist", bufs=1) as persist,
            tc.tile_pool(name="smalls", bufs=1) as smalls,
            tc.tile_pool(name="qpool", bufs=2) as qpool,
            tc.tile_pool(name="xp", bufs=2) as xp,
            tc.tile_pool(name="otp", bufs=3) as otp,
            tc.tile_pool(name="ep", bufs=10) as ep,
            tc.tile_pool(name="op", bufs=4) as op,
            tc.tile_pool(name="oap", bufs=8) as oap,
            tc.tile_pool(name="npool", bufs=3) as npool,
            tc.tile_pool(name="psS", bufs=2, space="PSUM") as psS,
            tc.tile_pool(name="psO", bufs=2, space="PSUM") as psO,
            tc.tile_pool(name="ps3", bufs=2, space="PSUM") as ps3p,
        ):
            # per-(chunk, slice) tiles: Tile's dependency tracking is
            # coarse-grained, so slices written by different projection
            # chains must be separate tiles or chunk-c attention reads
            # falsely serialize on later slices' writes.
            kT_t = {}  # (c, hp) -> [feature-partition, token]
            vx_t = {}  # (c, tt) -> [token-partition, head, d]
            for c in range(NTCH):
                for sl in range(NHP):
                    kT_t[(c, sl)] = persist.tile([P, TCH], bdt, name=f"kT{c}_{sl}")
                    vx_t[(c, sl)] = persist.tile(
                        [P, HG, D], edt, name=f"vx{c}_{sl}"
                    )

            wqts = persist.tile([P, NHP, 8, P], bdt, name="wqts")
            wkts = persist.tile([P, NHP, 8, P], bdt, name="wkts")
            wvts = persist.tile([P, 8, CG], bdt, name="wvts")
            wots = persist.tile([P, NHP, C], bdt, name="wots")
            biases = smalls.tile([P, 2 * NHP + CG], f32, name="biases")
            bqs = biases[:, 0:NHP]
            bks = biases[:, NHP : 2 * NHP]
            bvbs = biases[:, 2 * NHP :]

            xtt0 = xp.tile([P, 8, TCH], bdt, tag="xtt", name="xtt0")

            # ---- input loads: large image DMAs in 3 phases so the critical
            # x0+Wq pieces get the full HBM bandwidth (queued DMAs round-robin
            # at packet granularity, so without phase barriers everything
            # finishes together at the end).  Phase barriers are tiny
            # SBUF->DRAM reads spanning both halves of the previous phase:
            # the barrier DMA instruction blocks its HWDGE ring until they
            # land.  Barrier target rows of `out` are rewritten later by the
            # real output projection.
            # Input loads, strictly phased: concurrently-queued DMAs
            # round-robin at packet granularity across all SDMA engines and
            # finish together, so each subsequent load is gated behind the
            # previous phase with a tiny DVE copy into its destination tile
            # (a WAW dependency both the scheduler and hardware respect).
            # Phase 0 is the minimal first-projection set: bias + x0a + wq-a.
            # ---- warmup + mask/identity setup: emitted BEFORE the DMA gate
            # chain so these DVE ops aren't stuck behind gate copies that
            # wait on DMA completions (the DVE queue is FIFO).
            wu = smalls.tile([P, P], bdt, name="wu")
            nc.vector.memset(wu, 0.125)
            wups = ps3p.tile([P, 64], f32, tag="pso3", name="wups")
            for i in range(100):
                nc.tensor.matmul(
                    wups, wu, wu[:, 0:64], start=(i == 0), stop=(i == 99)
                )
            wue = smalls.tile([P, 1], f32, name="wue")
            nc.scalar.activation(wue, wups[:, 0:1], Exp, scale=SCALE)

            # triangular causal mask for diagonal 128x128 blocks:
            # keep (j, q) where q >= j  (partition j, free q)
            cmask = smalls.tile([P, P], edt, name="cmask")
            nc.vector.tensor_scalar(
                cmask, wu, 0.0, 1.0, mybir.AluOpType.mult, mybir.AluOpType.add
            )
            nc.gpsimd.affine_select(
                out=cmask,
                in_=cmask,
                compare_op=mybir.AluOpType.is_ge,
                fill=0.0,
                base=0,
                pattern=[[1, P]],
                channel_multiplier=-1,
            )
            # identity matrix (bf16) for the final partial-sum matmul-add
            ident = smalls.tile([P, P], bdt, name="ident")
            nc.vector.tensor_scalar(
                ident, wu, 0.0, 1.0, mybir.AluOpType.mult, mybir.AluOpType.add
            )
            nc.gpsimd.affine_select(
                out=ident,
                in_=ident,
                compare_op=mybir.AluOpType.is_equal,
                fill=0.0,
                base=0,
                pattern=[[1, P]],
                channel_multiplier=-1,
            )

            def gate(dst, src):
                nc.vector.tensor_copy(dst, src)

            # phase 1: bias + x0 (sync ring) | wq + wk-slice01 (scalar ring).
            nc.sync.dma_start(biases, bias_im.ap())
            nc.sync.dma_start(xtt0[:, 0:4, :], xt_im.ap()[:, 0, 0:4, :])
            nc.sync.dma_start(xtt0[:, 4:8, :], xt_im.ap()[:, 0, 4:8, :])
            nc.scalar.dma_start(wqts[:, 0:2], wq_im.ap()[:, 0:2])
            nc.scalar.dma_start(wqts[:, 2:4], wq_im.ap()[:, 2:4])
            nc.scalar.dma_start(wkts[:, 0:2], wk_im.ap()[:, 0:2])
            # phase 2 (gated behind phase 1): wk-b | wv | wot
            gate(wkts[0:1, 2:4, 0, 0:8], xtt0[0:1, 3:5, 0:8])
            nc.sync.dma_start(wkts[:, 2:4], wk_im.ap()[:, 2:4])
            gate(wvts[0:1, 0:2, 0:8], wqts[0:1, 1:3, 0, 0:8])
            nc.scalar.dma_start(wvts[:, 0:4, :], wv_im.ap()[:, 0:4, :])
            nc.scalar.dma_start(wvts[:, 4:8, :], wv_im.ap()[:, 4:8, :])
            gate(wots[0:1, 0:2, 0:8], wvts[0:1, 3:5, 0:8])
            nc.scalar.dma_start(wots, wot_im.ap())

            qT_cur = {}  # chunk -> qT tile (2-buf rotation)
            xtt_cur = {0: xtt0}
            ot_t = {}

            # ---- pump queues: generators emitting proj/outproj matmuls in
            # ~2-matmul units so they weave between exp-paced attention ops.
            # `pending` (urgent: q/k/v projections - the attention stream
            # stalls without them) drains before `filler` (output
            # projections - pure latency-tolerant PE filler).
            pending = []
            filler = []

            def pump2(n):
                while n > 0:
                    q = pending if pending else filler
                    if not q:
                        return
                    tag, g = q[0]
                    try:
                        next(g)
                        n -= 1
                    except StopIteration:
                        q.pop(0)

            def drain_all():
                while pending or filler:
                    pump2(100)

            def ensure_done(tag):
                """Emission barrier: a consumer of `tag`'s outputs is about
                to be emitted; Tile tracks dependencies by emission order,
                so the producer must be fully emitted first."""
                while any(t == tag for t, _ in pending):
                    pump2(4)

            def qk_gen(c, sl):
                """q and k feature-slice sl of chunk c; yields every 2 mms."""
                qT_cur[(c, sl)] = qpool.tile(
                    [P, TCH], bdt, tag=f"qT{sl}", name=f"qT{c}_{sl}"
                )
                xtt = xtt_cur[c]
                psq = ps3p.tile([P, TCH], f32, tag="pso3", name=f"psq{c}_{sl}")
                for cs in range(8):
                    nc.tensor.matmul(
                        psq,
                        wqts[:, sl, cs, :],
                        xtt[:, cs, :],
                        start=(cs == 0),
                        stop=(cs == 7),
                    )
                    if cs % 2 == 1:
                        yield
                with tc.high_priority(offset=300_000):
                    nc.vector.tensor_scalar_add(
                        qT_cur[(c, sl)], psq, bqs[:, sl : sl + 1]
                    )
                psk = ps3p.tile([P, TCH], f32, tag="pso3", name=f"psk{c}_{sl}")
                for cs in range(8):
                    nc.tensor.matmul(
                        psk,
                        wkts[:, sl, cs, :],
                        xtt[:, cs, :],
                        start=(cs == 0),
                        stop=(cs == 7),
                    )
                    if cs % 2 == 1:
                        yield
                with tc.high_priority(offset=300_000):
                    nc.vector.tensor_scalar_add(
                        kT_t[(c, sl)], psk, bks[:, sl : sl + 1]
                    )

            def v_gen(c, sl):
                """v token-tile sl of chunk c; yields every 2 mms."""
                # ones column of v_ext (softmax-denominator trick);
                # memset can't write float32r -> DVE 0*x + 1.
                nc.vector.tensor_scalar(
                    vx_t[(c, sl)][:, :, D],
                    bvbs[:, 0:HG],
                    0.0,
                    1.0,
                    mybir.AluOpType.mult,
                    mybir.AluOpType.add,
                )
                xtt = xtt_cur[c]
                psv = ps3p.tile([P, CG], f32, tag="pso3", name=f"psv{c}_{sl}")
                for cs in range(8):
                    nc.tensor.matmul(
                        psv,
                        xtt[:, cs, P * sl : P * (sl + 1)],
                        wvts[:, cs, :],
                        start=(cs == 0),
                        stop=(cs == 7),
                    )
                    if cs % 2 == 1:
                        yield
                with tc.high_priority(offset=300_000):
                    nc.vector.tensor_add(
                        vx_t[(c, sl)][:, :, 0:D],
                        psv.rearrange("p (h d) -> p h d", d=D),
                        bvbs.rearrange("p (h d) -> p h d", d=D),
                    )

            def outproj_gen(ic, plo=0, phi=2 * (TCH // P)):
                otn = [ot_t[(ic, hp)] for hp in range(NHP)]
                for piece in range(plo, phi):
                    tt, mi = piece // 2, piece % 2
                    trow = TCH * ic + P * tt
                    if True:
                        msl = slice(TCH * mi, TCH * (mi + 1))
                        pso3 = ps3p.tile(
                            [P, TCH], f32, tag="pso3", name=f"ps3{ic}_{tt}_{mi}"
                        )
                        for hp in range(NHP):
                            nc.tensor.matmul(
                                pso3,
                                otn[hp][:, P * tt : P * (tt + 1)],
                                wots[:, hp, msl],
                                start=(hp == 0),
                                stop=(hp == NHP - 1),
                            )
                            if hp % 2 == 1:
                                yield
                        osb = op.tile(
                            [P, TCH], bdt, tag="osb", name=f"ob{ic}_{tt}_{mi}"
                        )
                        nc.vector.tensor_copy(osb, pso3)
                        nc.sync.dma_start(out_ap[trow : trow + P, msl], osb)

            obA = []

            def outproj_final_A():
                """Final chunk head-pairs 0-2 -> bf16 partials (runs during
                the last head-pair's attention)."""
                ic = NTCH - 1
                for tt in range(TCH // P):
                    for mi in range(C // TCH):
                        msl = slice(TCH * mi, TCH * (mi + 1))
                        pso3 = ps3p.tile(
                            [P, TCH], f32, tag="pso3", name=f"psA{tt}_{mi}"
                        )
                        for hp in (0, 1, 2):
                            nc.tensor.matmul(
                                pso3,
                                ot_t[(ic, hp)][:, P * tt : P * (tt + 1)],
                                wots[:, hp, msl],
                                start=(hp == 0),
                                stop=(hp == 2),
                            )
                            if hp == 1:
                                yield
                        ob = oap.tile(
                            [P, TCH], bdt, tag="obA", name=f"obA{2 * tt + mi}"
                        )
                        nc.vector.tensor_copy(ob, pso3)
                        obA.append(ob)
                        yield

            def outproj_final_B():
                """Tail: head-pair 3 matmul + identity partial-add; psum
                evacuation on the scalar engine (idle at the tail)."""
                ic = NTCH - 1
                otn3 = ot_t.pop((ic, 3))
                with tc.high_priority(offset=450_000):
                    for tt in range(TCH // P):
                        trow = TCH * ic + P * tt
                        for mi in range(C // TCH):
                            i = 2 * tt + mi
                            msl = slice(TCH * mi, TCH * (mi + 1))
                            pso3 = ps3p.tile(
                                [P, TCH], f32, tag="pso3", name=f"psB{i}"
                            )
                            nc.tensor.matmul(
                                pso3, ident, obA[i], start=True, stop=False
                            )
                            nc.tensor.matmul(
                                pso3,
                                otn3[:, P * tt : P * (tt + 1)],
                                wots[:, 3, msl],
                                start=False,
                                stop=True,
                            )
                            osb = op.tile(
                                [P, TCH], bdt, tag="osb", name=f"obf{i}"
                            )
                            nc.scalar.copy(osb, pso3)
                            nc.sync.dma_start(out_ap[trow : trow + P, msl], osb)

            def normalize(c, hp, oraw, final=False):
                # rows 0..63 are O^T, row 64 the softmax sums.
                # partition_broadcast only reads physical partition 0
                # (base-64 APs return garbage on HW): DMA-hop the row.
                off = 450_000 if final else 0
                otn = ot_t[(c, hp)]
                for h01 in range(2):
                    ow = oraw[h01]
                    stmp = npool.tile(
                        [1, TCH], f32, tag="stmp", name=f"st{c}_{hp}_{h01}"
                    )
                    rb = npool.tile(
                        [D, TCH], f32, tag="rb", name=f"rb{c}_{hp}_{h01}"
                    )
                    if final:
                        with tc.high_priority(offset=off):
                            nc.sync.dma_start(stmp, ow[D : D + 1, :])
                            nc.gpsimd.partition_broadcast(rb, stmp)
                            nc.vector.reciprocal_approx_fast(rb, rb)
                    else:
                        nc.sync.dma_start(stmp, ow[D : D + 1, :])
                        nc.gpsimd.partition_broadcast(rb, stmp)
                        nc.vector.reciprocal_approx_fast(rb, rb)
                    if h01 == 0:
                        if final:
                            with tc.high_priority(offset=off):
                                nc.vector.tensor_mul(otn[0:D, :], ow[0:D, :], rb)
                        else:
                            nc.vector.tensor_mul(otn[0:D, :], ow[0:D, :], rb)
                    else:
                        tmpn = npool.tile(
                            [D, TCH], bdt, tag="tmpn", name=f"tn{c}_{hp}"
                        )
                        if final:
                            with tc.high_priority(offset=off):
                                nc.vector.tensor_mul(tmpn, ow[0:D, :], rb)
                                nc.sync.dma_start(otn[D:P, :], tmpn)
                        else:
                            nc.vector.tensor_mul(tmpn, ow[0:D, :], rb)
                            nc.sync.dma_start(otn[D:P, :], tmpn)

            # ---- flat software-pipelined attention over all j-steps.
            steps = [
                (c, hp, jt)
                for c in range(NTCH)
                for hp in range(NHP)
                for jt in range(4 * (c + 1))
            ]
            LAG = 2
            ee_of = {}
            po_of = {}

            def passA(i):
                c, hp, jt = steps[i]
                if jt == 0:
                    ensure_done(f"qk{c}_{hp}")
                cj, lj = jt // 4, jt % 4
                r = jt - 4 * c
                lo = 0 if r <= 0 else P * r
                psp = psS.tile(
                    [P, 2, TCH], f32, tag="psp", name=f"psp{c}_{hp}_{jt}"
                )
                with tc.high_priority(offset=1_000_000):
                    for h01 in range(2):
                        pb = 64 * h01
                        nc.tensor.matmul(
                            psp[:, h01, lo:],
                            kT_t[(cj, hp)][pb : pb + D, P * lj : P * (lj + 1)],
                            qT_cur[(c, hp)][pb : pb + D, lo:],
                            start=True,
                            stop=True,
                        )
                    ee = ep.tile(
                        [P, 2, TCH], edt, tag="ee", name=f"ee{c}_{hp}_{jt}"
                    )
                    if r <= 0:
                        nc.scalar.activation(ee, psp, Exp, scale=SCALE)
                    else:
                        nc.scalar.activation(
                            ee[:, :, lo:], psp[:, :, lo:], Exp, scale=SCALE
                        )
                    if r >= 0:
                        bsl = slice(P * r, P * (r + 1))
                        for h01 in range(2):
                            nc.vector.tensor_mul(
                                ee[:, h01, bsl], ee[:, h01, bsl], cmask
                            )
                ee_of[i] = ee

            def passB(j):
                c, hp, jt = steps[j]
                njt = 4 * (c + 1)
                if jt == 0:
                    ot_t[(c, hp)] = otp.tile(
                        [P, TCH], bdt, tag=f"ot{hp}", name=f"ot{c}_{hp}"
                    )
                    po_of[(c, hp)] = (
                        psO.tile([P, TCH], f32, tag="ps_o", name=f"poe{c}_{hp}"),
                        psO.tile([P, TCH], f32, tag="ps_o", name=f"poo{c}_{hp}"),
                    )
                ps_os = po_of[(c, hp)]
                ee = ee_of.pop(j)
                cj, lj = jt // 4, jt % 4
                if cj == c:
                    ensure_done(f"v{c}_{lj}")
                lo = max(0, P * (jt - 4 * c))
                with tc.high_priority(offset=500_000):
                    for h01 in range(2):
                        nc.tensor.matmul(
                            ps_os[h01][0 : D + 1, lo:],
                            vx_t[(cj, lj)][:, 2 * hp + h01, :],
                            ee[:, h01, lo:],
                            start=(jt == 0),
                            stop=(jt == njt - 1),
                        )
                if jt == njt - 1:
                    del po_of[(c, hp)]
                    oraw = []
                    for h01 in range(2):
                        ow = npool.tile(
                            [D + 1, TCH],
                            f32,
                            tag=f"oraw{h01}",
                            name=f"or{c}_{hp}_{h01}",
                        )
                        with tc.high_priority(offset=500_000):
                            nc.vector.tensor_copy(ow, ps_os[h01][0 : D + 1, :])
                        oraw.append(ow)
                    final = c == NTCH - 1 and hp == NHP - 1
                    if final:
                        # held-back output-projection pieces: pure PE filler
                        # released at the tail so the PE has work while the
                        # final normalize's latency chain runs.
                        filler.append(("out2b", outproj_gen(NTCH - 2, 5, 8)))
                    normalize(c, hp, oraw, final=final)
                    if hp == NHP - 1 and c < NTCH - 1:
                        phi = 5 if c == NTCH - 2 else 8
                        filler.append((f"out{c}", outproj_gen(c, 0, phi)))
                    if c == NTCH - 1 and hp == 2:
                        filler.append(("outA", outproj_final_A()))

            # proj generator launch schedule, keyed by global head-pair
            # phase ph = 4c + hp.  Each qk(c, sl) is queued ~2 phases before
            # its attention phase needs it: early enough that the pump
            # finishes it in time, late enough that chunks 2-3 (exp-heavy)
            # keep their own share of PE filler.
            schedule = {0: [("qk0_2", qk_gen, (0, 2))], 1: [("qk0_3", qk_gen, (0, 3))]}
            for c in range(1, NTCH):
                schedule.setdefault(4 * c - 2, []).extend(
                    [
                        (f"qk{c}_0", qk_gen, (c, 0)),
                        (f"v{c}_0", v_gen, (c, 0)),
                        (f"v{c}_1", v_gen, (c, 1)),
                    ]
                )
                schedule.setdefault(4 * c - 1, []).extend(
                    [
                        (f"qk{c}_1", qk_gen, (c, 1)),
                        (f"v{c}_2", v_gen, (c, 2)),
                        (f"v{c}_3", v_gen, (c, 3)),
                    ]
                )
                schedule.setdefault(4 * c, []).append((f"qk{c}_2", qk_gen, (c, 2)))
                schedule.setdefault(4 * c + 1, []).append(
                    (f"qk{c}_3", qk_gen, (c, 3))
                )

            # proj(0) slice 0 inline; the S/exp stream starts immediately
            # after so the scheduler places it at the head of the PE queue.
            for _ in qk_gen(0, 0):
                pass
            for sl in range(NHP):
                pending.append((f"v0_{sl}", v_gen(0, sl)))
            pending.append(("qk0_1", qk_gen(0, 1)))

            for i in range(len(steps) + LAG):
                if i < len(steps):
                    c, hp, jt = steps[i]
                    if jt == 0:
                        ph = 4 * c + hp
                        if hp == 0 and c + 1 < NTCH:
                            # prefetch next chunk's x (gated behind wk for
                            # chunk 1 so it can't steal phase-1 bandwidth)
                            xtt = xp.tile(
                                [P, 8, TCH], bdt, tag="xtt", name=f"xtt{c + 1}"
                            )
                            if c == 0:
                                nc.vector.tensor_copy(
                                    xtt[0:1, 0:2, 0:8], wkts[0:1, 2:4, 0, 0:8]
                                )
                            nc.sync.dma_start(xtt, xt_im.ap()[:, c + 1])
                            xtt_cur[c + 1] = xtt
                        for tag, fn, args in schedule.pop(ph, []):
                            pending.append((tag, fn(*args)))
                    passA(i)
                    pump2(2)
                if i >= LAG:
                    passB(i - LAG)
                    pump2(2)

            drain_all()
            outproj_final_B()

    nc.compile()
    return nc


def get_module(mm_fast=True):
    key = bool(mm_fast)
    if key not in _MODULE_CACHE:
        _MODULE_CACHE[key] = _build_module(key)
    return _MODULE_CACHE[key]


def make_in_maps(x, Wq, bq, Wk, bk, Wv, bv, Wo, bo):
    import ml_dtypes

    bf16 = ml_dtypes.bfloat16
    x = np.asarray(x, dtype=np.float32)
    Wq = np.asarray(Wq, dtype=np.float32)
    Wk = np.asarray(Wk, dtype=np.float32)
    Wv = np.asarray(Wv, dtype=np.float32)
    Wo = np.asarray(Wo, dtype=np.float32)
    bq = np.asarray(bq, dtype=np.float32)
    bk = np.asarray(bk, dtype=np.float32)
    bv = np.asarray(bv, dtype=np.float32)

    in_maps = []
    for core in range(NCORES):
        b, g = core // GROUPS, core % GROUPS
        gs = slice(CG * g, CG * (g + 1))
        # x image: [p, chunk, cs, t']
        xim = (
            x[b].T.reshape(8, P, NTCH, TCH).transpose(1, 2, 0, 3)
        )  # [128, 4, 8, 512]
        # wq/wk image: [p, j4, cs, j']
        wqi = Wq[gs, :].T.reshape(8, P, NHP, P).transpose(1, 2, 0, 3)
        wki = Wk[gs, :].T.reshape(8, P, NHP, P).transpose(1, 2, 0, 3)
        # wv image: [p, cs, j]
        wvi = Wv[gs, :].T.reshape(8, P, CG).transpose(1, 0, 2)
        # wot image: [p, hp, m]
        woi = Wo[:, gs].T.reshape(NHP, P, C).transpose(1, 0, 2)
        bias = np.concatenate(
            [
                bq[gs].reshape(NHP, P).T,
                bk[gs].reshape(NHP, P).T,
                np.broadcast_to(bv[gs][None, :], (P, CG)),
            ],
            axis=1,
        )
        in_maps.append(
            {
                "xt_im": np.ascontiguousarray(xim).astype(bf16),
                "wq_im": np.ascontiguousarray(wqi).astype(bf16),
                "wk_im": np.ascontiguousarray(wki).astype(bf16),
                "wv_im": np.ascontiguousarray(wvi).astype(bf16),
                "wot_im": np.ascontiguousarray(woi).astype(bf16),
                "bias_im": np.ascontiguousarray(bias),
            }
        )
    return in_maps


def combine_results(results, bo):
    bo = np.asarray(bo, dtype=np.float32)
    out = np.empty((B, T, C), dtype=np.float32)
    for b in range(B):
        out[b] = (
            results[GROUPS * b]["out"].astype(np.float32)
            + results[GROUPS * b + 1]["out"].astype(np.float32)
            + bo[None, :]
        )
    return out


def kernel(**inputs):
    from concourse.bass_utils import run_bass_kernel_spmd

    nc = get_module(mm_fast=True)
    in_maps = make_in_maps(
        inputs["x"],
        inputs["Wq"],
        inputs["bq"],
        inputs["Wk"],
        inputs["bk"],
        inputs["Wv"],
        inputs["bv"],
        inputs["Wo"],
        inputs["bo"],
    )
    res = run_bass_kernel_spmd(nc, in_maps, core_ids=list(range(NCORES)))
    return combine_results(res.results, inputs["bo"])
